# revision 62
# baseline (speedup 1.0000x reference)
"""Trainium2 Bass kernel for Gemma4Audio chunked local attention.

Sharding: 8 cores = batch(4) x seq-half(2). Each core processes 3072
tokens of one batch (plus a 12-token left halo for k/v context) fully
locally -- block-local attention never crosses the half-sequence
boundary mid-block, so no collectives are needed.

Per-core algorithm (features-on-partition [d, token] layout):
  - q/k/v projections as W-chunk.T @ x matmuls (bf16 x and weights,
    f32 PSUM accumulation)
  - per 96-query tile: scores = q.T @ k over an exact 108-key window in
    f16 (full PE rate), rel-position scores via two small matmuls
    (normal q and q shifted one token for the rel_shift row leak)
  - one GPSIMD local_scatter places rel scores diagonally into window
    coords AND writes the -30000 band mask (mask values are constant
    columns of the scatter source)
  - softmax: DVE adds scatter output into score PSUM, ACT tanh softcap,
    ACT exp into bf16 (e^50 fits bf16, so no max-subtraction), DVE
    per-tile row-sum + reciprocal, normalize into f16 (DVE/GPSIMD split)
  - PE transposes attn via f16 identity; V is transposed by the DMA
    xbar (dma_start_transpose) off the critical path; AV matmul; post
    projection from an f16 attn-out buffer.
Software pipeline: slot i runs proj(i) | scores+softmax(i-1) |
attn-transpose(i-3) | AV(i-4), with the post projection of region r-1
spread over three head-slots of region r, so the tensor engine never
waits on the softmax chain.
"""

import math

import numpy as np

# ---- model constants (hardcoded per problem spec) ----
HID = 1024
H = 8
D = 128
CHUNK = 12
PAST = 12
CTX = 24
P = 25
SOFTCAP = 50.0
Q_SCALE = D ** (-0.5) / math.log(2)
K_SCALE = math.log(1.0 + math.e) / math.log(2)

B = 4
S = 6144
NCORES = 8

T = S // 2          # 3072 tokens per core
THALO = T + PAST    # 3084 with left halo
TR = 384            # tokens per region
NREG = T // TR      # 8
TPB = 96            # queries per attention tile
NTILE = TR // TPB   # 4
W = TPB + CHUNK     # 108-key window per tile
KC = HID // 128     # 8 contraction chunks
DFREE = 112         # dst window stride (>= W, even)
NELEMS = NTILE * DFREE          # 448 scatter dest elems
NIDX = 4 * 52 + 4 * 88          # 560: bd slots + mask slots
MASKVAL = -30000.0

_CACHE = {}


def _build_idx():
    """Scatter index table: slots [g*64+p] main rel, [g*64+32+p] prev-query
    rel (rel_shift leak), [256+g*88+j] constant mask writes covering every
    out-of-band column, so the scatter output is the complete rel+mask term."""
    idx = np.full((128, NIDX), -1, dtype=np.int16)
    for a in range(TPB):
        i, c = divmod(a, CHUNK)
        for g in range(NTILE):
            base = g * DFREE
            for p in range(P):
                if c + p < CTX:
                    idx[a, g * 52 + p] = base + a + p
                if p >= P - c:
                    idx[a, g * 52 + 26 + p] = base + a + p - P
            j = 0
            for w in range(DFREE):
                if not (CHUNK * i <= w < CHUNK * i + CTX):
                    idx[a, 208 + g * 88 + j] = base + w
                    j += 1
            assert j == 88
    return idx


def _build_bass():
    import concourse.bass as bass
    import concourse.bacc as bacc
    import concourse.mybir as mybir
    import concourse.tile as tile

    dt = mybir.dt
    f32 = dt.float32
    f32r = dt.float32r
    f16 = dt.float16
    bf16 = dt.bfloat16
    AF = mybir.ActivationFunctionType
    ALU = mybir.AluOpType
    AX = mybir.AxisListType

    nc = bacc.Bacc(None, target_bir_lowering=False)

    xT = nc.declare_dram_parameter("xT", [128, KC * THALO], bf16, isOutput=False)
    wqT = nc.declare_dram_parameter("wqT", [128, KC * HID], bf16, isOutput=False)
    wkT = nc.declare_dram_parameter("wkT", [128, KC * HID], bf16, isOutput=False)
    wvT = nc.declare_dram_parameter("wvT", [128, KC * HID], bf16, isOutput=False)
    wpT = nc.declare_dram_parameter("wpT", [128, KC * HID], f16, isOutput=False)
    relkT = nc.declare_dram_parameter("relkT", [128, H * 32], f16, isOutput=False)
    identT = nc.declare_dram_parameter("identT", [128, 128], f16, isOutput=False)
    idxT = nc.declare_dram_parameter("idxT", [128, NIDX], dt.int16, isOutput=False)
    outT = nc.declare_dram_parameter("outT", [HID, T], f32, isOutput=True)

    import contextlib

    with tile.TileContext(nc) as tc:
        with contextlib.ExitStack() as ctx:
            def pool(name, bufs, space=None):
                kw = {"space": space} if space else {}
                return ctx.enter_context(tc.tile_pool(name=name, bufs=bufs, **kw))

            cpool = pool("consts", 1)
            wpool = pool("weights", 1)
            xpool = pool("xin", 2)
            qpool = pool("qsb", 4)
            kvpool = pool("kvsb", 4)
            dpool = pool("dat", 3)
            dstpool = pool("dstp", 3)
            thpool = pool("thp", 3)
            atpool = pool("atp", 3)
            statpool = pool("statp", 3)
            atvsbpool = pool("atvsb", 3)
            vtpool = pool("vtsb", 5)
            aopool = pool("ao", 2)
            opool = pool("outsb", 3)
            pqpool = pool("psPQ", 2, "PSUM")
            kvpspool = pool("psKV", 1, "PSUM")
            vpspool = pool("psKV2", 1, "PSUM")
            sallpool = pool("psSA", 1, "PSUM")
            bdpool = pool("psBD", 1, "PSUM")
            atvpool = pool("psATV", 1, "PSUM")
            aopspool = pool("psAO", 1, "PSUM")
            env = dict(
                nc=nc, mybir=mybir, cpool=cpool, wpool=wpool, xpool=xpool,
                qpool=qpool, kvpool=kvpool, dpool=dpool, dstpool=dstpool,
                thpool=thpool, atpool=atpool, statpool=statpool,
                atvsbpool=atvsbpool, vtpool=vtpool, aopool=aopool, opool=opool,
                pqpool=pqpool, kvpspool=kvpspool, vpspool=vpspool,
                sallpool=sallpool,
                bdpool=bdpool, atvpool=atvpool, aopspool=aopspool,
                xT=xT, wqT=wqT, wkT=wkT, wvT=wvT, wpT=wpT, relkT=relkT,
                identT=identT, idxT=idxT, outT=outT,
                f32=f32, f32r=f32r, f16=f16, bf16=bf16, AF=AF, ALU=ALU, AX=AX,
                dt=dt,
            )
            _main(env)
    nc.compile()
    return nc


def _main(env):
    (nc, mybir, cpool, wpool, xpool, qpool, kvpool, dpool, dstpool, thpool,
     atpool, statpool, atvsbpool, vtpool, aopool, opool, pqpool, kvpspool, vpspool, sallpool,
     bdpool, atvpool, aopspool, xT, wqT, wkT, wvT, wpT, relkT, identT, idxT,
     outT, f32, f32r, f16, bf16, AF, ALU, AX, dt) = (
        env[k] for k in (
            "nc", "mybir", "cpool", "wpool", "xpool", "qpool", "kvpool",
            "dpool", "dstpool", "thpool", "atpool", "statpool", "atvsbpool",
            "vtpool", "aopool", "opool", "pqpool", "kvpspool", "vpspool", "sallpool", "bdpool",
            "atvpool", "aopspool", "xT", "wqT", "wkT", "wvT", "wpT", "relkT",
            "identT", "idxT", "outT", "f32", "f32r", "f16", "bf16", "AF",
            "ALU", "AX", "dt",
        )
    )
    if True:
        if True:
            # --- x region 0 + first weight chunks first: projections can
            # start as soon as (xr0, wq kc0) land.
            xv = xT.rearrange("p (kc n) -> p kc n", n=THALO)
            xr0 = xpool.tile([128, KC, TR + PAST], bf16, tag="xr")
            nc.sync.dma_start(out=xr0[:], in_=xv[:, :, 0 : TR + PAST])
            wq_sb = wpool.tile([128, KC, HID], bf16, tag="wq")
            wk_sb = wpool.tile([128, KC, HID], bf16, tag="wk")
            wv_sb = wpool.tile([128, KC, HID], bf16, tag="wv")
            # stream q/k/v weights by head-pair so head 0's projections
            # can start ~6us in, while later head groups arrive in the
            # background
            for hp in range(2):
                cs = slice(hp * 512, (hp + 1) * 512)
                for wsb, wdr in ((wq_sb, wqT), (wk_sb, wkT), (wv_sb, wvT)):
                    wv3 = wdr.rearrange("p (kc o) -> p kc o", o=HID)
                    nc.sync.dma_start(out=wsb[:, :, cs], in_=wv3[:, :, cs])
                if hp == 0:
                    # small constants ride along after the first chunks
                    idx_sb = cpool.tile([128, NIDX], dt.int16, tag="idx")
                    nc.sync.dma_start(out=idx_sb[:], in_=idxT[:, :])
                    relk_sb = cpool.tile([128, H, 32], f16, tag="relk")
                    nc.sync.dma_start(out=relk_sb[:], in_=relkT[:, :])
                    ident = cpool.tile([128, 128], f16, tag="ident")
                    nc.sync.dma_start(out=ident[:], in_=identT[:, :])
            wp_sb = wpool.tile([128, KC, HID], f16, tag="wp")
            nc.sync.dma_start(out=wp_sb[:], in_=wpT[:, :])

            # mask constant columns of the two rotating scatter-data bufs
            for _ in range(3):
                dinit = dpool.tile([128, NIDX], f16, tag="data")
                nc.vector.memset(dinit[:, 208:NIDX], MASKVAL)
            # zero the v-pad columns of the rotating kv buffers (read by the
            # 128-wide DMA transposes, never used downstream)
            for _ in range(4):
                kvinit = kvpool.tile([128, 2, 416], f16, tag="kv")
                nc.vector.memset(kvinit[:, 1, 396:416], 0.0)

            st = {"prevA": None, "prevB": None, "prevC": None, "prevD": None,
                  "xr": xr0, "aoT": {}, "pocnt": 0}

            def proj_q(r, h):
                hs = slice(h * 128, (h + 1) * 128)
                xr = st["xr"]
                qps = pqpool.tile([128, 386], f32, tag="pq")
                for kc in range(KC):
                    nc.tensor.matmul(
                        qps[:], lhsT=wq_sb[:, kc, hs], rhs=xr[:, kc, 10:396],
                        start=kc == 0, stop=kc == KC - 1,
                    )
                q_sb = qpool.tile([128, 386], f16, tag="q")
                kv_sb = kvpool.tile([128, 2, 416], f16, tag="kv")
                nc.scalar.copy(q_sb[:], qps[:])
                return dict(r=r, h=h, q=q_sb, kv=kv_sb, xr=xr)

            def proj_k(sl, kvps):
                h, xr, kv_sb = sl["h"], sl["xr"], sl["kv"]
                hs = slice(h * 128, (h + 1) * 128)
                for kc in range(KC):
                    nc.tensor.matmul(
                        kvps[:, 0, 0:396], lhsT=wk_sb[:, kc, hs], rhs=xr[:, kc, :],
                        start=kc == 0, stop=kc == KC - 1,
                    )
                nc.scalar.copy(kv_sb[:, 0, 0:396], kvps[:, 0, 0:396])

            def proj_v(sl, kvps):
                h, xr, kv_sb = sl["h"], sl["xr"], sl["kv"]
                hs = slice(h * 128, (h + 1) * 128)
                for kc in range(KC):
                    nc.tensor.matmul(
                        kvps[:, 1, 0:396], lhsT=wv_sb[:, kc, hs], rhs=xr[:, kc, :],
                        start=kc == 0, stop=kc == KC - 1,
                    )
                nc.scalar.copy(kv_sb[:, 1, 0:396], kvps[:, 1, 0:396])
                vt_sb = vtpool.tile([128, NTILE, 128], f16, tag="vt")
                for g in range(NTILE):
                    b0 = TPB * g
                    nc.sync.dma_start_transpose(
                        out=vt_sb[:, g, :], in_=kv_sb[:, 1, b0 : b0 + 128]
                    )
                sl["vt"] = vt_sb

            def attnA_pe(sl):
                h, q_sb, kv_sb = sl["h"], sl["q"], sl["kv"]
                sall = sallpool.tile([96, NTILE, W], f32, tag="sall")
                bdall = bdpool.tile([96, NTILE, 52], f32, tag="bd")
                for g in range(NTILE):
                    b0 = TPB * g
                    qmain = q_sb[:, b0 + 2 : b0 + 2 + TPB]
                    qprev = q_sb[:, b0 + 1 : b0 + 1 + TPB]
                    nc.tensor.matmul(
                        sall[:, g, :], lhsT=qmain, rhs=kv_sb[:, 0, b0 : b0 + W],
                        start=True, stop=True,
                    )
                    nc.tensor.matmul(
                        bdall[:, g, 0:26], lhsT=qmain, rhs=relk_sb[:, h, 0:26],
                        start=True, stop=True,
                    )
                    nc.tensor.matmul(
                        bdall[:, g, 26:52], lhsT=qprev, rhs=relk_sb[:, h, 0:26],
                        start=True, stop=True,
                    )
                sl["sall"] = sall
                sl["bdall"] = bdall

            def attnA_rest(sl):
                sall, bdall = sl["sall"], sl["bdall"]
                data = dpool.tile([128, NIDX], f16, tag="data")
                nc.vector.tensor_copy(data[0:96, 0:208], bdall[:, :, :])
                dst = dstpool.tile([128, NELEMS], f16, tag="dst")
                nc.gpsimd.local_scatter(
                    dst[0:96, :], data[0:96, :], idx_sb[0:96, :],
                    channels=96, num_elems=NELEMS, num_idxs=NIDX,
                )
                nc.vector.tensor_tensor(
                    out=sall[:], in0=sall[:],
                    in1=dst[0:96, :].rearrange("p (g w) -> p g w", g=NTILE)[:, :, 0:W],
                    op=ALU.add,
                )
                th = thpool.tile([96, NTILE, W], f16, tag="th")
                nc.scalar.activation(
                    out=th[:], in_=sall[:], func=AF.Tanh, scale=1.0 / SOFTCAP
                )
                # exp in bf16: e^50 fits bf16 range, so no max-subtraction
                ate = atpool.tile([96, NTILE, W], bf16, tag="ate")
                nc.scalar.activation(
                    out=ate[:], in_=th[:], func=AF.Exp, scale=SOFTCAP
                )
                stat = statpool.tile([96, 8], f32, tag="stat")
                nc.vector.tensor_reduce(
                    out=stat[:, 4:8], in_=ate[:], op=ALU.add, axis=AX.X
                )
                nc.vector.reciprocal(stat[:, 4:8], stat[:, 4:8])
                at = atpool.tile([96, NTILE, W], f16, tag="at")
                for g in range(NTILE):
                    eng = nc.vector if g % 2 == 0 else nc.gpsimd
                    eng.tensor_scalar_mul(
                        out=at[:, g, :], in0=ate[:, g, :],
                        scalar1=stat[:, 4 + g : 5 + g],
                    )
                sl["at"] = at

            def attnB1(sl):
                at = sl["at"]
                atv_ps = atvpool.tile([W, NTILE, TPB], f16, tag="atv")
                for g in range(NTILE):
                    nc.tensor.transpose(
                        atv_ps[:, g, :], at[:, g, :], ident[0:96, 0:96]
                    )
                atv_sb = atvsbpool.tile([W, NTILE, TPB], f16, tag="atvsb")
                nc.vector.tensor_copy(atv_sb[:], atv_ps[:])
                sl["atv"] = atv_sb

            def attnB2(sl):
                r, h, atv_sb, vt_sb = sl["r"], sl["h"], sl["atv"], sl["vt"]
                aops = aopspool.tile([128, NTILE, TPB], f32, tag="aops")
                for g in range(NTILE):
                    nc.tensor.matmul(
                        aops[:, g, :], lhsT=vt_sb[0:W, g, :],
                        rhs=atv_sb[:, g, :], start=True, stop=True,
                    )
                aoT = st["aoT"][r]
                nc.vector.tensor_copy(
                    aoT[:, h, :], aops[:].rearrange("p g w -> p (g w)")
                )

            def post(r, ocs):
                aoT = st["aoT"][r]
                for oc in ocs:
                    pps = pqpool.tile([128, 386], f32, tag="pq")
                    for h in range(H):
                        nc.tensor.matmul(
                            pps[:, 0:TR],
                            lhsT=wp_sb[:, h, oc * 128 : (oc + 1) * 128],
                            rhs=aoT[:, h, :],
                            start=(h == 0), stop=(h == H - 1),
                        )
                    po = opool.tile([128, TR], f32, tag="po")
                    if st["pocnt"] % 2 == 0:
                        nc.vector.tensor_copy(po[:], pps[:, 0:TR])
                    else:
                        nc.scalar.copy(po[:], pps[:, 0:TR])
                    st["pocnt"] += 1
                    nc.sync.dma_start(
                        out=outT[oc * 128 : (oc + 1) * 128, r * TR : (r + 1) * TR],
                        in_=po[:],
                    )

            # --- software-pipelined main loop: per-slot tensor-engine
            # order is [scores(i-1) | AV(i-2) | proj(i)] so the softmax
            # chain of i-1 overlaps proj(i) fully.
            PHASED_R0 = False
            if PHASED_R0:
                sl0 = [proj_q(0, h) for h in range(H)]
                for h in range(H):
                    kvps0 = kvpspool.tile([128, 2, 512], f32, tag="kv",
                                          name="kvps0")
                    proj_k(sl0[h], kvps0)
            for r in range(NREG):
                st["aoT"][r] = aopool.tile([128, H, TR], f16, tag="aoT",
                                           name="aoT")
                for h in range(H):
                    if r == 0 and PHASED_R0:
                        cur = sl0[h]
                        kvps = kvpspool.tile([128, 2, 512], f32, tag="kv",
                                             name="kvps")
                        proj_v(cur, kvps)
                    else:
                        cur = proj_q(r, h)
                        kvps = kvpspool.tile([128, 2, 512], f32, tag="kv",
                                             name="kvps")
                        proj_k(cur, kvps)
                        proj_v(cur, kvps)
                    if st["prevA"] is not None:
                        attnA_pe(st["prevA"])
                        attnA_rest(st["prevA"])
                    if st["prevC"] is not None:
                        attnB1(st["prevC"])
                    if st["prevD"] is not None:
                        attnB2(st["prevD"])
                    if h == 0 and r + 1 < NREG:
                        xr = xpool.tile([128, KC, TR + PAST], bf16, tag="xr")
                        nc.sync.dma_start(
                            out=xr[:],
                            in_=xv[:, :, (r + 1) * TR : (r + 1) * TR + TR + PAST],
                        )
                        st["xr_next"] = xr
                    st["prevD"] = st["prevC"]
                    st["prevC"] = st["prevB"]
                    st["prevB"] = st["prevA"]
                    st["prevA"] = cur
                    if h == 4 and r > 0:
                        post(r - 1, range(0, 2))
                    if h == 5 and r > 0:
                        post(r - 1, range(2, 4))
                    if h == 6 and r > 0:
                        post(r - 1, range(4, 6))
                    if h == 7 and r > 0:
                        post(r - 1, range(6, 8))
                if r + 1 < NREG:
                    st["xr"] = st["xr_next"]
            attnA_pe(st["prevA"])
            attnA_rest(st["prevA"])
            for sl in (st["prevD"], st["prevC"], st["prevB"], st["prevA"]):
                if "atv" not in sl:
                    attnB1(sl)
                attnB2(sl)
            post(NREG - 1, range(KC))


def _get_nc():
    if "nc" not in _CACHE:
        _CACHE["nc"] = _build_bass()
    return _CACHE["nc"]


def _pack(w):
    # [1024 in, 1024 out] -> [128, kc*1024] partition-major
    return np.ascontiguousarray(
        w.reshape(KC, 128, HID).transpose(1, 0, 2).reshape(128, KC * HID)
    )


def _prepare_in_maps(hidden_states, position_embeddings, Wq, Wk, Wv, Wpost, Wrel,
                     per_dim_scale):
    import ml_dtypes

    f32 = np.float32
    hs = np.asarray(hidden_states, f32)
    pe = np.asarray(position_embeddings, np.float64)
    qscale = Q_SCALE * np.log1p(np.exp(np.asarray(per_dim_scale, np.float64)))
    qs_tiled = np.tile(qscale, H)
    wqT = _pack((np.asarray(Wq, np.float64) * qs_tiled[:, None]).T.astype(f32)).astype(ml_dtypes.bfloat16)
    wkT = _pack((np.asarray(Wk, np.float64) * K_SCALE).T.astype(f32)).astype(ml_dtypes.bfloat16)
    wvT = _pack(np.asarray(Wv, f32).T).astype(ml_dtypes.bfloat16)
    wpT = _pack(np.asarray(Wpost, f32).T).astype(np.float16)

    # host-side rel_k = pos_emb @ Wrel.T  -> [128, h*32] f16 (cols 25:32 zero)
    relk = (pe @ np.asarray(Wrel, np.float64).T).astype(f32)  # [P, HID]
    relk3 = np.zeros((128, H, 32), f32)
    relk3[:, :, :P] = relk.reshape(P, H, 128).transpose(2, 1, 0)
    relkT = np.ascontiguousarray(relk3.reshape(128, H * 32)).astype(np.float16)

    identT = np.eye(128, dtype=np.float16)
    idx = _build_idx()

    shared = dict(wqT=wqT, wkT=wkT, wvT=wvT, wpT=wpT, relkT=relkT,
                  identT=identT, idxT=idx)
    in_maps = []
    for core in range(NCORES):
        b, half = divmod(core, 2)
        lo = half * T
        slab = np.zeros((THALO, HID), f32)
        src_lo = max(lo - PAST, 0)
        slab[PAST - (lo - src_lo) :, :] = hs[b, src_lo : lo + T, :]
        xT = np.ascontiguousarray(
            slab.T.reshape(KC, 128, THALO).transpose(1, 0, 2).reshape(128, KC * THALO)
        ).astype(ml_dtypes.bfloat16)
        in_maps.append(dict(xT=xT, **shared))
    return in_maps


def _assemble(results):
    out = np.empty((B, S, HID), np.float32)
    for core in range(NCORES):
        b, half = divmod(core, 2)
        out[b, half * T : (half + 1) * T, :] = results[core]["outT"].T
    return out


def kernel(**inputs) -> np.ndarray:
    from concourse.bass_utils import run_bass_kernel_spmd

    nc = _get_nc()
    in_maps = _prepare_in_maps(**inputs)
    res = run_bass_kernel_spmd(nc, in_maps, list(range(NCORES)))
    return _assemble(res.results)


# revision 63
# speedup vs baseline: 1.0010x; 1.0010x over previous
"""Trainium2 Bass kernel for Gemma4Audio chunked local attention.

Sharding: 8 cores = batch(4) x seq-half(2). Each core processes 3072
tokens of one batch (plus a 12-token left halo for k/v context) fully
locally -- block-local attention never crosses the half-sequence
boundary mid-block, so no collectives are needed.

Per-core algorithm (features-on-partition [d, token] layout):
  - q/k/v projections as W-chunk.T @ x matmuls (bf16 x and weights,
    f32 PSUM accumulation)
  - per 96-query tile: scores = q.T @ k over an exact 108-key window in
    f16 (full PE rate), rel-position scores via two small matmuls
    (normal q and q shifted one token for the rel_shift row leak)
  - one GPSIMD local_scatter places rel scores diagonally into window
    coords AND writes the -30000 band mask (mask values are constant
    columns of the scatter source)
  - softmax: DVE adds scatter output into score PSUM, ACT tanh softcap,
    ACT exp into bf16 (e^50 fits bf16, so no max-subtraction), DVE
    per-tile row-sum + reciprocal, normalize into f16 (DVE/GPSIMD split)
  - PE transposes attn via f16 identity; V is transposed by the DMA
    xbar (dma_start_transpose) off the critical path; AV matmul; post
    projection from an f16 attn-out buffer.
Software pipeline: slot i runs proj(i) | scores+softmax(i-1) |
attn-transpose(i-3) | AV(i-4), with the post projection of region r-1
spread over three head-slots of region r, so the tensor engine never
waits on the softmax chain.
"""

import math

import numpy as np

# ---- model constants (hardcoded per problem spec) ----
HID = 1024
H = 8
D = 128
CHUNK = 12
PAST = 12
CTX = 24
P = 25
SOFTCAP = 50.0
Q_SCALE = D ** (-0.5) / math.log(2)
K_SCALE = math.log(1.0 + math.e) / math.log(2)

B = 4
S = 6144
NCORES = 8

T = S // 2          # 3072 tokens per core
THALO = T + PAST    # 3084 with left halo
TR = 384            # tokens per region
NREG = T // TR      # 8
TPB = 96            # queries per attention tile
NTILE = TR // TPB   # 4
W = TPB + CHUNK     # 108-key window per tile
KC = HID // 128     # 8 contraction chunks
DFREE = 112         # dst window stride (>= W, even)
NELEMS = NTILE * DFREE          # 448 scatter dest elems
NIDX = 4 * 52 + 4 * 88          # 560: bd slots + mask slots
MASKVAL = -30000.0

_CACHE = {}


def _build_idx():
    """Scatter index table: slots [g*64+p] main rel, [g*64+32+p] prev-query
    rel (rel_shift leak), [256+g*88+j] constant mask writes covering every
    out-of-band column, so the scatter output is the complete rel+mask term."""
    idx = np.full((128, NIDX), -1, dtype=np.int16)
    for a in range(TPB):
        i, c = divmod(a, CHUNK)
        for g in range(NTILE):
            base = g * DFREE
            for p in range(P):
                if c + p < CTX:
                    idx[a, g * 52 + p] = base + a + p
                if p >= P - c:
                    idx[a, g * 52 + 26 + p] = base + a + p - P
            j = 0
            for w in range(DFREE):
                if not (CHUNK * i <= w < CHUNK * i + CTX):
                    idx[a, 208 + g * 88 + j] = base + w
                    j += 1
            assert j == 88
    return idx


def _build_bass():
    import concourse.bass as bass
    import concourse.bacc as bacc
    import concourse.mybir as mybir
    import concourse.tile as tile

    dt = mybir.dt
    f32 = dt.float32
    f32r = dt.float32r
    f16 = dt.float16
    bf16 = dt.bfloat16
    AF = mybir.ActivationFunctionType
    ALU = mybir.AluOpType
    AX = mybir.AxisListType

    nc = bacc.Bacc(None, target_bir_lowering=False)

    xT = nc.declare_dram_parameter("xT", [128, KC * THALO], bf16, isOutput=False)
    wqT = nc.declare_dram_parameter("wqT", [128, KC * HID], bf16, isOutput=False)
    wkT = nc.declare_dram_parameter("wkT", [128, KC * HID], bf16, isOutput=False)
    wvT = nc.declare_dram_parameter("wvT", [128, KC * HID], bf16, isOutput=False)
    wpT = nc.declare_dram_parameter("wpT", [128, KC * HID], f16, isOutput=False)
    relkT = nc.declare_dram_parameter("relkT", [128, H * 32], f16, isOutput=False)
    identT = nc.declare_dram_parameter("identT", [128, 128], f16, isOutput=False)
    idxT = nc.declare_dram_parameter("idxT", [128, NIDX], dt.int16, isOutput=False)
    outT = nc.declare_dram_parameter("outT", [HID, T], f32, isOutput=True)

    import contextlib

    with tile.TileContext(nc) as tc:
        with contextlib.ExitStack() as ctx:
            def pool(name, bufs, space=None):
                kw = {"space": space} if space else {}
                return ctx.enter_context(tc.tile_pool(name=name, bufs=bufs, **kw))

            cpool = pool("consts", 1)
            wpool = pool("weights", 1)
            xpool = pool("xin", 2)
            qpool = pool("qsb", 4)
            kvpool = pool("kvsb", 4)
            dpool = pool("dat", 3)
            dstpool = pool("dstp", 3)
            thpool = pool("thp", 3)
            atpool = pool("atp", 3)
            statpool = pool("statp", 3)
            atvsbpool = pool("atvsb", 3)
            vtpool = pool("vtsb", 5)
            aopool = pool("ao", 2)
            opool = pool("outsb", 3)
            pqpool = pool("psPQ", 2, "PSUM")
            kvpspool = pool("psKV", 1, "PSUM")
            vpspool = pool("psKV2", 1, "PSUM")
            sallpool = pool("psSA", 1, "PSUM")
            bdpool = pool("psBD", 1, "PSUM")
            atvpool = pool("psATV", 1, "PSUM")
            aopspool = pool("psAO", 1, "PSUM")
            env = dict(
                nc=nc, mybir=mybir, cpool=cpool, wpool=wpool, xpool=xpool,
                qpool=qpool, kvpool=kvpool, dpool=dpool, dstpool=dstpool,
                thpool=thpool, atpool=atpool, statpool=statpool,
                atvsbpool=atvsbpool, vtpool=vtpool, aopool=aopool, opool=opool,
                pqpool=pqpool, kvpspool=kvpspool, vpspool=vpspool,
                sallpool=sallpool,
                bdpool=bdpool, atvpool=atvpool, aopspool=aopspool,
                xT=xT, wqT=wqT, wkT=wkT, wvT=wvT, wpT=wpT, relkT=relkT,
                identT=identT, idxT=idxT, outT=outT,
                f32=f32, f32r=f32r, f16=f16, bf16=bf16, AF=AF, ALU=ALU, AX=AX,
                dt=dt,
            )
            _main(env)
    nc.compile()
    return nc


def _main(env):
    (nc, mybir, cpool, wpool, xpool, qpool, kvpool, dpool, dstpool, thpool,
     atpool, statpool, atvsbpool, vtpool, aopool, opool, pqpool, kvpspool, vpspool, sallpool,
     bdpool, atvpool, aopspool, xT, wqT, wkT, wvT, wpT, relkT, identT, idxT,
     outT, f32, f32r, f16, bf16, AF, ALU, AX, dt) = (
        env[k] for k in (
            "nc", "mybir", "cpool", "wpool", "xpool", "qpool", "kvpool",
            "dpool", "dstpool", "thpool", "atpool", "statpool", "atvsbpool",
            "vtpool", "aopool", "opool", "pqpool", "kvpspool", "vpspool", "sallpool", "bdpool",
            "atvpool", "aopspool", "xT", "wqT", "wkT", "wvT", "wpT", "relkT",
            "identT", "idxT", "outT", "f32", "f32r", "f16", "bf16", "AF",
            "ALU", "AX", "dt",
        )
    )
    if True:
        if True:
            # --- x region 0 + first weight chunks first: projections can
            # start as soon as (xr0, wq kc0) land.
            xv = xT.rearrange("p (kc n) -> p kc n", n=THALO)
            xr0 = xpool.tile([128, KC, TR + PAST], bf16, tag="xr")
            nc.sync.dma_start(out=xr0[:], in_=xv[:, :, 0 : TR + PAST])
            wq_sb = wpool.tile([128, KC, HID], bf16, tag="wq")
            wk_sb = wpool.tile([128, KC, HID], bf16, tag="wk")
            wv_sb = wpool.tile([128, KC, HID], bf16, tag="wv")
            # stream q/k/v weights by head-pair so head 0's projections
            # can start ~6us in, while later head groups arrive in the
            # background
            for hp in range(2):
                cs = slice(hp * 512, (hp + 1) * 512)
                for wsb, wdr in ((wq_sb, wqT), (wk_sb, wkT), (wv_sb, wvT)):
                    wv3 = wdr.rearrange("p (kc o) -> p kc o", o=HID)
                    nc.sync.dma_start(out=wsb[:, :, cs], in_=wv3[:, :, cs])
                if hp == 0:
                    # small constants ride along after the first chunks
                    idx_sb = cpool.tile([128, NIDX], dt.int16, tag="idx")
                    nc.sync.dma_start(out=idx_sb[:], in_=idxT[:, :])
                    relk_sb = cpool.tile([128, H, 32], f16, tag="relk")
                    nc.sync.dma_start(out=relk_sb[:], in_=relkT[:, :])
                    ident = cpool.tile([128, 128], f16, tag="ident")
                    nc.sync.dma_start(out=ident[:], in_=identT[:, :])
            wp_sb = wpool.tile([128, KC, HID], f16, tag="wp")
            nc.sync.dma_start(out=wp_sb[:], in_=wpT[:, :])

            # mask constant columns of the two rotating scatter-data bufs
            for _ in range(3):
                dinit = dpool.tile([128, NIDX], f16, tag="data")
                nc.vector.memset(dinit[:, 208:NIDX], MASKVAL)
            # zero the v-pad columns of the rotating kv buffers (read by the
            # 128-wide DMA transposes, never used downstream)
            for _ in range(4):
                kvinit = kvpool.tile([128, 2, 416], f16, tag="kv")
                nc.vector.memset(kvinit[:, 1, 396:416], 0.0)

            st = {"prevA": None, "prevB": None, "prevC": None, "prevD": None,
                  "xr": xr0, "aoT": {}, "pocnt": 0}

            def proj_q(r, h):
                hs = slice(h * 128, (h + 1) * 128)
                xr = st["xr"]
                qps = pqpool.tile([128, 386], f32, tag="pq")
                for kc in range(KC):
                    nc.tensor.matmul(
                        qps[:], lhsT=wq_sb[:, kc, hs], rhs=xr[:, kc, 10:396],
                        start=kc == 0, stop=kc == KC - 1,
                    )
                q_sb = qpool.tile([128, 386], f16, tag="q")
                kv_sb = kvpool.tile([128, 2, 416], f16, tag="kv")
                nc.scalar.copy(q_sb[:], qps[:])
                return dict(r=r, h=h, q=q_sb, kv=kv_sb, xr=xr)

            def proj_k(sl, kvps):
                h, xr, kv_sb = sl["h"], sl["xr"], sl["kv"]
                hs = slice(h * 128, (h + 1) * 128)
                for kc in range(KC):
                    nc.tensor.matmul(
                        kvps[:, 0, 0:396], lhsT=wk_sb[:, kc, hs], rhs=xr[:, kc, :],
                        start=kc == 0, stop=kc == KC - 1,
                    )
                nc.scalar.copy(kv_sb[:, 0, 0:396], kvps[:, 0, 0:396])

            def proj_v(sl, kvps):
                h, xr, kv_sb = sl["h"], sl["xr"], sl["kv"]
                hs = slice(h * 128, (h + 1) * 128)
                for kc in range(KC):
                    nc.tensor.matmul(
                        kvps[:, 1, 0:396], lhsT=wv_sb[:, kc, hs], rhs=xr[:, kc, :],
                        start=kc == 0, stop=kc == KC - 1,
                    )
                nc.scalar.copy(kv_sb[:, 1, 0:396], kvps[:, 1, 0:396])
                vt_sb = vtpool.tile([128, NTILE, 128], f16, tag="vt")
                for g in range(NTILE):
                    b0 = TPB * g
                    nc.sync.dma_start_transpose(
                        out=vt_sb[:, g, :], in_=kv_sb[:, 1, b0 : b0 + 128]
                    )
                sl["vt"] = vt_sb

            def attnA_pe(sl):
                h, q_sb, kv_sb = sl["h"], sl["q"], sl["kv"]
                sall = sallpool.tile([96, NTILE, W], f32, tag="sall")
                bdall = bdpool.tile([96, NTILE, 52], f32, tag="bd")
                for g in range(NTILE):
                    b0 = TPB * g
                    qmain = q_sb[:, b0 + 2 : b0 + 2 + TPB]
                    qprev = q_sb[:, b0 + 1 : b0 + 1 + TPB]
                    nc.tensor.matmul(
                        sall[:, g, :], lhsT=qmain, rhs=kv_sb[:, 0, b0 : b0 + W],
                        start=True, stop=True,
                    )
                    nc.tensor.matmul(
                        bdall[:, g, 0:26], lhsT=qmain, rhs=relk_sb[:, h, 0:26],
                        start=True, stop=True,
                    )
                    nc.tensor.matmul(
                        bdall[:, g, 26:52], lhsT=qprev, rhs=relk_sb[:, h, 0:26],
                        start=True, stop=True,
                    )
                sl["sall"] = sall
                sl["bdall"] = bdall

            def attnA_rest(sl):
                sall, bdall = sl["sall"], sl["bdall"]
                data = dpool.tile([128, NIDX], f16, tag="data")
                nc.vector.tensor_copy(data[0:96, 0:208], bdall[:, :, :])
                dst = dstpool.tile([128, NELEMS], f16, tag="dst")
                nc.gpsimd.local_scatter(
                    dst[0:96, :], data[0:96, :], idx_sb[0:96, :],
                    channels=96, num_elems=NELEMS, num_idxs=NIDX,
                )
                nc.vector.tensor_tensor(
                    out=sall[:], in0=sall[:],
                    in1=dst[0:96, :].rearrange("p (g w) -> p g w", g=NTILE)[:, :, 0:W],
                    op=ALU.add,
                )
                th = thpool.tile([96, NTILE, W], f16, tag="th")
                nc.scalar.activation(
                    out=th[:], in_=sall[:], func=AF.Tanh, scale=1.0 / SOFTCAP
                )
                # exp in bf16: e^50 fits bf16 range, so no max-subtraction
                ate = atpool.tile([96, NTILE, W], bf16, tag="ate")
                nc.scalar.activation(
                    out=ate[:], in_=th[:], func=AF.Exp, scale=SOFTCAP
                )
                stat = statpool.tile([96, 8], f32, tag="stat")
                nc.vector.tensor_reduce(
                    out=stat[:, 4:8], in_=ate[:], op=ALU.add, axis=AX.X
                )
                nc.vector.reciprocal(stat[:, 4:8], stat[:, 4:8])
                at = atpool.tile([96, NTILE, W], f16, tag="at")
                for g in range(NTILE):
                    eng = nc.vector
                    eng.tensor_scalar_mul(
                        out=at[:, g, :], in0=ate[:, g, :],
                        scalar1=stat[:, 4 + g : 5 + g],
                    )
                sl["at"] = at

            def attnB1(sl):
                at = sl["at"]
                atv_ps = atvpool.tile([W, NTILE, TPB], f16, tag="atv")
                for g in range(NTILE):
                    nc.tensor.transpose(
                        atv_ps[:, g, :], at[:, g, :], ident[0:96, 0:96]
                    )
                atv_sb = atvsbpool.tile([W, NTILE, TPB], f16, tag="atvsb")
                nc.vector.tensor_copy(atv_sb[:], atv_ps[:])
                sl["atv"] = atv_sb

            def attnB2(sl):
                r, h, atv_sb, vt_sb = sl["r"], sl["h"], sl["atv"], sl["vt"]
                aops = aopspool.tile([128, NTILE, TPB], f32, tag="aops")
                for g in range(NTILE):
                    nc.tensor.matmul(
                        aops[:, g, :], lhsT=vt_sb[0:W, g, :],
                        rhs=atv_sb[:, g, :], start=True, stop=True,
                    )
                aoT = st["aoT"][r]
                nc.vector.tensor_copy(
                    aoT[:, h, :], aops[:].rearrange("p g w -> p (g w)")
                )

            def post(r, ocs):
                aoT = st["aoT"][r]
                for oc in ocs:
                    pps = pqpool.tile([128, 386], f32, tag="pq")
                    for h in range(H):
                        nc.tensor.matmul(
                            pps[:, 0:TR],
                            lhsT=wp_sb[:, h, oc * 128 : (oc + 1) * 128],
                            rhs=aoT[:, h, :],
                            start=(h == 0), stop=(h == H - 1),
                        )
                    po = opool.tile([128, TR], f32, tag="po")
                    if st["pocnt"] % 2 == 0:
                        nc.vector.tensor_copy(po[:], pps[:, 0:TR])
                    else:
                        nc.scalar.copy(po[:], pps[:, 0:TR])
                    st["pocnt"] += 1
                    nc.sync.dma_start(
                        out=outT[oc * 128 : (oc + 1) * 128, r * TR : (r + 1) * TR],
                        in_=po[:],
                    )

            # --- software-pipelined main loop: per-slot tensor-engine
            # order is [scores(i-1) | AV(i-2) | proj(i)] so the softmax
            # chain of i-1 overlaps proj(i) fully.
            PHASED_R0 = False
            if PHASED_R0:
                sl0 = [proj_q(0, h) for h in range(H)]
                for h in range(H):
                    kvps0 = kvpspool.tile([128, 2, 512], f32, tag="kv",
                                          name="kvps0")
                    proj_k(sl0[h], kvps0)
            for r in range(NREG):
                st["aoT"][r] = aopool.tile([128, H, TR], f16, tag="aoT",
                                           name="aoT")
                for h in range(H):
                    if r == 0 and PHASED_R0:
                        cur = sl0[h]
                        kvps = kvpspool.tile([128, 2, 512], f32, tag="kv",
                                             name="kvps")
                        proj_v(cur, kvps)
                    else:
                        cur = proj_q(r, h)
                        kvps = kvpspool.tile([128, 2, 512], f32, tag="kv",
                                             name="kvps")
                        proj_k(cur, kvps)
                        proj_v(cur, kvps)
                    if st["prevA"] is not None:
                        attnA_pe(st["prevA"])
                        attnA_rest(st["prevA"])
                    if st["prevC"] is not None:
                        attnB1(st["prevC"])
                    if st["prevD"] is not None:
                        attnB2(st["prevD"])
                    if h == 0 and r + 1 < NREG:
                        xr = xpool.tile([128, KC, TR + PAST], bf16, tag="xr")
                        nc.sync.dma_start(
                            out=xr[:],
                            in_=xv[:, :, (r + 1) * TR : (r + 1) * TR + TR + PAST],
                        )
                        st["xr_next"] = xr
                    st["prevD"] = st["prevC"]
                    st["prevC"] = st["prevB"]
                    st["prevB"] = st["prevA"]
                    st["prevA"] = cur
                    if h == 4 and r > 0:
                        post(r - 1, range(0, 2))
                    if h == 5 and r > 0:
                        post(r - 1, range(2, 4))
                    if h == 6 and r > 0:
                        post(r - 1, range(4, 6))
                    if h == 7 and r > 0:
                        post(r - 1, range(6, 8))
                if r + 1 < NREG:
                    st["xr"] = st["xr_next"]
            attnA_pe(st["prevA"])
            attnA_rest(st["prevA"])
            for sl in (st["prevD"], st["prevC"], st["prevB"], st["prevA"]):
                if "atv" not in sl:
                    attnB1(sl)
                attnB2(sl)
            post(NREG - 1, range(KC))


def _get_nc():
    if "nc" not in _CACHE:
        _CACHE["nc"] = _build_bass()
    return _CACHE["nc"]


def _pack(w):
    # [1024 in, 1024 out] -> [128, kc*1024] partition-major
    return np.ascontiguousarray(
        w.reshape(KC, 128, HID).transpose(1, 0, 2).reshape(128, KC * HID)
    )


def _prepare_in_maps(hidden_states, position_embeddings, Wq, Wk, Wv, Wpost, Wrel,
                     per_dim_scale):
    import ml_dtypes

    f32 = np.float32
    hs = np.asarray(hidden_states, f32)
    pe = np.asarray(position_embeddings, np.float64)
    qscale = Q_SCALE * np.log1p(np.exp(np.asarray(per_dim_scale, np.float64)))
    qs_tiled = np.tile(qscale, H)
    wqT = _pack((np.asarray(Wq, np.float64) * qs_tiled[:, None]).T.astype(f32)).astype(ml_dtypes.bfloat16)
    wkT = _pack((np.asarray(Wk, np.float64) * K_SCALE).T.astype(f32)).astype(ml_dtypes.bfloat16)
    wvT = _pack(np.asarray(Wv, f32).T).astype(ml_dtypes.bfloat16)
    wpT = _pack(np.asarray(Wpost, f32).T).astype(np.float16)

    # host-side rel_k = pos_emb @ Wrel.T  -> [128, h*32] f16 (cols 25:32 zero)
    relk = (pe @ np.asarray(Wrel, np.float64).T).astype(f32)  # [P, HID]
    relk3 = np.zeros((128, H, 32), f32)
    relk3[:, :, :P] = relk.reshape(P, H, 128).transpose(2, 1, 0)
    relkT = np.ascontiguousarray(relk3.reshape(128, H * 32)).astype(np.float16)

    identT = np.eye(128, dtype=np.float16)
    idx = _build_idx()

    shared = dict(wqT=wqT, wkT=wkT, wvT=wvT, wpT=wpT, relkT=relkT,
                  identT=identT, idxT=idx)
    in_maps = []
    for core in range(NCORES):
        b, half = divmod(core, 2)
        lo = half * T
        slab = np.zeros((THALO, HID), f32)
        src_lo = max(lo - PAST, 0)
        slab[PAST - (lo - src_lo) :, :] = hs[b, src_lo : lo + T, :]
        xT = np.ascontiguousarray(
            slab.T.reshape(KC, 128, THALO).transpose(1, 0, 2).reshape(128, KC * THALO)
        ).astype(ml_dtypes.bfloat16)
        in_maps.append(dict(xT=xT, **shared))
    return in_maps


def _assemble(results):
    out = np.empty((B, S, HID), np.float32)
    for core in range(NCORES):
        b, half = divmod(core, 2)
        out[b, half * T : (half + 1) * T, :] = results[core]["outT"].T
    return out


def kernel(**inputs) -> np.ndarray:
    from concourse.bass_utils import run_bass_kernel_spmd

    nc = _get_nc()
    in_maps = _prepare_in_maps(**inputs)
    res = run_bass_kernel_spmd(nc, in_maps, list(range(NCORES)))
    return _assemble(res.results)
